# revision 20
# baseline (speedup 1.0000x reference)
"""Trainium2 Bass kernel: atrous (dilated) multi-head attention block.

Computation (per reference):
  x [2, 4096, 1024] --atrous regroup (dil=4)--> xr [8, 1024, 1024]
  q/k/v = xr @ W{q,k,v}.T + b;  16 heads, dh=64
  probs = softmax(q k^T / 8);  ctx = probs v
  atted = ctx @ Wf.T + bf;  final = LN(atted + x)
  returns (final, atted)

Sharding: B*dil == 8 == n_cores, so each NeuronCore takes one atrous group
[1024, 1024] — pure data parallel, zero collectives. The host performs the
strided regroup/scatter (that IS the shard selection) and pre-transposes /
pre-scales operands so the device kernel needs no on-chip transposes.

Hardware facts this schedule is built around (measured):
  - 512-col bf16 matmul streams at 216ns start-to-start ONLY when
    consecutive matmuls accumulate into the same psum group; alternating
    psum tiles with an LDWEIGHTS in between halves the rate.  So every
    K-chunk accumulation chain is emitted as one contiguous same-tile run.
  - ScalarE ACTIVATE costs (N+352)/1.2GHz; the 128 exp instructions are
    ~147us, so everything else is kept off ScalarE where possible
    (atted psum copies -> DVE, LN square -> GPSIMD).
  - Inputs/weights are per-chunk tiles so the first V-proj matmul waits
    on chunk 0's DMA only, not the whole tensor.
  - bv is folded into bf on the host (softmax weights sum to 1), so the
    V drain is a pure ScalarE copy.
  - Both graded outputs leave as bf16 (host converts) to halve the output
    DMA tail.
"""

import os
import sys
from contextlib import ExitStack

for _p in ("/opt/trn_rl_repo",):
    if os.path.isdir(_p) and _p not in sys.path:
        sys.path.insert(0, _p)

import numpy as np
import ml_dtypes

import concourse.bass as bass
import concourse.mybir as mybir
from concourse.tile import TileContext
from concourse.bass_utils import run_bass_kernel_spmd

B, S, D = 2, 4096, 1024
DIL = 4
NCORES = 8
L = S // DIL  # 1024 rows per core
H, DH = 16, 64
P = 128
KC = D // P  # 8 contraction chunks
MT = D // P  # 8 output chunks
NT = 512  # matmul free-dim tile
EPS = 1e-5
SCALE = 1.0 / 8.0  # 1/sqrt(dh)

F32 = mybir.dt.float32
BF16 = mybir.dt.bfloat16
AL = mybir.AluOpType
AF = mybir.ActivationFunctionType
BF16_NP = ml_dtypes.bfloat16


def _split_excess_waits(nc: bass.Bass, max_waits: int = 1) -> None:
    """This neuronxcc's walrus rejects instructions carrying more than
    `max_waits` semaphore waits ("Too many sync wait commands").  Tile's
    kernel-tail drain (and occasionally a compute op) can exceed that.
    Move the excess waits onto same-engine no-ops inserted just before the
    instruction — the engine executes in order, so the happens-before
    relation is preserved exactly."""
    n = 0
    for fn in nc.m.functions:
        for blk in fn.blocks:
            insts = list(blk.instructions)
            out = []
            changed = False
            for inst in insts:
                si = inst.sync_info
                waits = list(si.on_wait) if (si is not None and si.on_wait) else []
                if len(waits) > max_waits:
                    changed = True
                    excess, keep = waits[:-max_waits], waits[-max_waits:]
                    for i in range(0, len(excess), max_waits):
                        nop = mybir.InstNoOp(name=f"waitsplit-{n}", ins=[], outs=[])
                        n += 1
                        nop.engine = inst.engine
                        nop.sync_info = mybir.SyncInfo(
                            on_wait=excess[i : i + max_waits], on_update=[]
                        )
                        nc.register_instruction(nop)
                        out.append(nop)
                    si.on_wait = keep
                out.append(inst)
            if changed:
                blk.instructions = out


def build_graph(apply_affine: bool = False) -> bass.Bass:
    nc = bass.Bass()
    xT_e = nc.declare_dram_parameter("xT", [D, L], BF16, isOutput=False)
    xn_e = nc.declare_dram_parameter("xn", [L, D], BF16, isOutput=False)
    wq_e = nc.declare_dram_parameter("wqT", [D, D], BF16, isOutput=False)
    wk_e = nc.declare_dram_parameter("wkT", [D, D], BF16, isOutput=False)
    wv_e = nc.declare_dram_parameter("wvT", [D, D], BF16, isOutput=False)
    wf_e = nc.declare_dram_parameter("wfT", [D, D], BF16, isOutput=False)
    bqc_e = nc.declare_dram_parameter("bqc", [P, MT], F32, isOutput=False)
    bkc_e = nc.declare_dram_parameter("bkc", [P, MT], F32, isOutput=False)
    bfh_e = nc.declare_dram_parameter("bfh", [D], BF16, isOutput=False)
    gam_e = nc.declare_dram_parameter("gam", [D], F32, isOutput=False)
    bet_e = nc.declare_dram_parameter("bet", [D], F32, isOutput=False)
    out_e = nc.declare_dram_parameter("out", [2, L, D], BF16, isOutput=True)

    wq_s = wq_e.rearrange("(kc p) n -> p kc n", p=P)
    wk_s = wk_e.rearrange("(kc p) n -> p kc n", p=P)
    wv_s = wv_e.rearrange("(kc p) n -> p kc n", p=P)
    wf_s = wf_e.rearrange("(kc p) n -> p kc n", p=P)
    xT_s = xT_e.rearrange("(kc p) l -> p kc l", p=P)

    with TileContext(nc) as tc, ExitStack() as ctx:
        const = ctx.enter_context(tc.tile_pool(name="const", bufs=1))
        persist = ctx.enter_context(tc.tile_pool(name="persist", bufs=1))
        wpool = ctx.enter_context(tc.tile_pool(name="wpool", bufs=24))
        epool = ctx.enter_context(tc.tile_pool(name="epool", bufs=3))
        mmps = ctx.enter_context(tc.tile_pool(name="mmps", bufs=2, space="PSUM"))
        cpool = ctx.enter_context(tc.tile_pool(name="cpool", bufs=2, space="PSUM"))
        spool = ctx.enter_context(tc.tile_pool(name="spool", bufs=2, space="PSUM"))
        rpool = ctx.enter_context(tc.tile_pool(name="rpool", bufs=2))
        dpool = ctx.enter_context(tc.tile_pool(name="dpool", bufs=2, space="DRAM"))
        xpool = ctx.enter_context(tc.tile_pool(name="xpool", bufs=4))
        opool = ctx.enter_context(tc.tile_pool(name="opool", bufs=2))
        stat = ctx.enter_context(tc.tile_pool(name="stat", bufs=4))

        # ---- interleaved per-chunk xT / wv DMAs; separate tiles so the
        # first V-proj matmul depends only on chunk 0 of each.
        xT_t = [persist.tile([P, L], BF16, tag=f"xT{kc}", name=f"xT{kc}") for kc in range(KC)]
        wv_t = [wpool.tile([P, D], BF16, tag="w", name=f"wv{kc}") for kc in range(KC)]
        for kc in range(KC):
            nc.sync.dma_start(out=xT_t[kc][:], in_=xT_s[:, kc, :])
            nc.sync.dma_start(out=wv_t[kc][:], in_=wv_s[:, kc, :])

        def load_w(src):
            ts = [wpool.tile([P, D], BF16, tag="w", name="w") for _ in range(KC)]
            for kc in range(KC):
                nc.sync.dma_start(out=ts[kc][:], in_=src[:, kc, :])
            return ts

        wq_t = load_w(wq_s)
        wk_t = load_w(wk_s)

        # ---- small constants
        bqc_sb = const.tile([P, MT], F32, tag="bqc")
        nc.sync.dma_start(out=bqc_sb[:], in_=bqc_e[:])
        bkc_sb = const.tile([P, MT], F32, tag="bkc")
        nc.sync.dma_start(out=bkc_sb[:], in_=bkc_e[:])
        ones_r = const.tile([1, P], BF16, tag="ones_r")
        nc.vector.memset(ones_r[:], 1.0)
        bfr = const.tile([1, D], BF16, tag="bfr")
        nc.sync.dma_start(out=bfr[:], in_=bfh_e[None, :])
        epsb = const.tile([P, 1], F32, tag="epsb")
        nc.vector.memset(epsb[:], EPS)
        if apply_affine:
            gmb = const.tile([P, D], F32, tag="gmb")
            nc.sync.dma_start(out=gmb[:], in_=gam_e[None, :].to_broadcast((P, D)))
            btb = const.tile([P, D], F32, tag="btb")
            nc.sync.dma_start(out=btb[:], in_=bet_e[None, :].to_broadcast((P, D)))

        # per-chunk persistent arrays (separate tiles => fine-grained deps)
        qT = [persist.tile([P, L], BF16, tag=f"qT{m}", name=f"qT{m}") for m in range(MT)]
        kT = [persist.tile([P, L], BF16, tag=f"kT{m}", name=f"kT{m}") for m in range(MT)]
        vA = [persist.tile([P, H, DH + 1], BF16, tag=f"vA{m}", name=f"vA{m}") for m in range(KC)]
        cT = [persist.tile([P, L], BF16, tag=f"cT{m}", name=f"cT{m}") for m in range(KC)]
        for m in range(KC):
            nc.vector.memset(vA[m][:, :, DH : DH + 1], 1.0)

        def v_drain(ps, m, t):
            # no bias (bv folded into bf_eff on host) -> pure copy + pack
            nc.scalar.activation(
                vA[m][:, t * 8 : (t + 1) * 8, 0:DH],
                ps[:].rearrange("p (h e) -> p h e", e=DH),
                AF.Copy,
            )

        # ---- V projection.
        # Phase A (m=0,1): kc-outer so the matmuls pace with the chunk DMAs
        # (starts ~1.5us in; runs DMA-bound, psum-alternation cost hidden).
        pssA = [mmps.tile([P, NT], F32, tag="mm", name=f"vA{i}") for i in range(2)] + [
            cpool.tile([P, NT], F32, tag="cx", name=f"vA{i+2}") for i in range(2)
        ]
        for kc in range(KC):
            for mi in range(2):
                for t in range(2):
                    nc.tensor.matmul(
                        pssA[2 * mi + t][:],
                        xT_t[kc][:, mi * P : (mi + 1) * P],
                        wv_t[kc][:, t * NT : (t + 1) * NT],
                        start=(kc == 0),
                        stop=(kc == KC - 1),
                    )
        for mi in range(2):
            for t in range(2):
                v_drain(pssA[2 * mi + t], mi, t)
        # Phase B (m=2..7): chunks are resident; contiguous same-tile runs.
        for m in range(2, MT):
            for t in range(2):
                ps = mmps.tile([P, NT], F32, tag="mm", name=f"vB{m}_{t}")
                for kc in range(KC):
                    nc.tensor.matmul(
                        ps[:],
                        xT_t[kc][:, m * P : (m + 1) * P],
                        wv_t[kc][:, t * NT : (t + 1) * NT],
                        start=(kc == 0),
                        stop=(kc == KC - 1),
                    )
                v_drain(ps, m, t)

        # wf replaces wv in the weight ring; needed only at F-proj.
        wf_t = load_w(wf_s)

        def emit_qk(m):
            for w_t, bias_sb, dst in ((wq_t, bqc_sb, qT), (wk_t, bkc_sb, kT)):
                for t in range(2):
                    ps = mmps.tile([P, NT], F32, tag="mm", name=f"qk{m}_{t}")
                    for kc in range(KC):
                        nc.tensor.matmul(
                            ps[:],
                            w_t[kc][:, m * P : (m + 1) * P],
                            xT_t[kc][:, t * NT : (t + 1) * NT],
                            start=(kc == 0),
                            stop=(kc == KC - 1),
                        )
                    # bias-add + bf16 cast on ScalarE
                    nc.scalar.activation(
                        dst[m][:, t * NT : (t + 1) * NT],
                        ps[:],
                        AF.Identity,
                        bias=bias_sb[:, m : m + 1],
                    )

        def emit_scores(h):
            """scoresT + exp for one head -> eT tile [j, jc, i]."""
            hc, hh = h // 2, h % 2
            hp = hh * DH
            eT = epool.tile([P, KC, L], BF16, tag="eT", name=f"eT{h}")
            for jc in range(KC):
                ps = spool.tile([P, L], F32, tag="sc", name=f"sc{h}_{jc}")
                for t in range(2):
                    nc.tensor.matmul(
                        ps[:, t * NT : (t + 1) * NT],
                        kT[hc][hp : hp + DH, jc * P : (jc + 1) * P],
                        qT[hc][hp : hp + DH, t * NT : (t + 1) * NT],
                        start=True,
                        stop=True,
                    )
                nc.scalar.activation(eT[:, jc, :], ps[:], AF.Exp)
            return eT

        def emit_ilv(h_sc, h_cx, eT_cx):
            """Interleave one head's scores+exp with another head's ctx at
            jc granularity: the PE is in-order, so during the exp-throttled
            scores stream (spool has 2 slots, exp drains at ~1.1us/tile) the
            ctx matmuls keep it busy.  Returns the new head's eT (or None).
            """
            eT = None
            if h_sc is not None:
                eT = epool.tile([P, KC, L], BF16, tag="eT", name=f"eT{h_sc}")
                sc_hc, sc_hh = h_sc // 2, h_sc % 2
                sc_hp = sc_hh * DH
            cx_hc, cx_hh = h_cx // 2, h_cx % 2
            cx_hp = cx_hh * DH
            pcs = [cpool.tile([P, NT], F32, tag="cx", name=f"pc{h_cx}_{t}") for t in range(2)]
            if h_sc is not None:
                for jc in range(KC):
                    ps = spool.tile([P, L], F32, tag="sc", name=f"sc{h_sc}_{jc}")
                    for t in range(2):
                        nc.tensor.matmul(
                            ps[:, t * NT : (t + 1) * NT],
                            kT[sc_hc][sc_hp : sc_hp + DH, jc * P : (jc + 1) * P],
                            qT[sc_hc][sc_hp : sc_hp + DH, t * NT : (t + 1) * NT],
                            start=True,
                            stop=True,
                        )
                    nc.scalar.activation(eT[:, jc, :], ps[:], AF.Exp)
            # t-major: 8 contiguous matmuls into one psum group keep the PE
            # at 216ns/mm (LDWEIGHTS hides only within a same-group run);
            # each vA chunk is loaded twice, but the loads are hidden.
            for t in range(2):
                for jc in range(KC):
                    nc.tensor.matmul(
                        pcs[t][0 : DH + 1, :],
                        vA[jc][:, h_cx, :],
                        eT_cx[:, jc, t * NT : (t + 1) * NT],
                        start=(jc == 0),
                        stop=(jc == KC - 1),
                    )
            # denominator chains, paired across the two i-halves so the DVE
            # queue isn't head-of-line blocked: both evacs first, then both
            # DMA reshape/broadcast pipelines run concurrently, then both
            # multiplies.  (DVE reciprocal is ~5 passes over the FREE dim,
            # so a [1,512] recip costs 2.7us while [128,4] costs ~0.2us —
            # hence the DRAM bounce to reshape across partitions.)
            cus = []
            for t in range(2):
                cu = rpool.tile([DH + 1, NT], F32, tag="cu")
                nc.vector.tensor_copy(out=cu[:], in_=pcs[t][0 : DH + 1, :])
                cus.append(cu)
            rbs = []
            for t in range(2):
                rdA = dpool.tile([1, NT], F32, tag="rdA")
                nc.sync.dma_start(out=rdA[:], in_=cus[t][DH : DH + 1, :])
                st = rpool.tile([P, NT // P], F32, tag="st")
                nc.sync.dma_start(
                    out=st[:], in_=rdA[0, :].rearrange("(p f) -> p f", p=P)
                )
                stR = rpool.tile([P, NT // P], F32, tag="stR")
                nc.vector.reciprocal(stR[:], st[:])
                rdB = dpool.tile([1, NT], F32, tag="rdB")
                nc.sync.dma_start(
                    out=rdB[0, :].rearrange("(p f) -> p f", p=P), in_=stR[:]
                )
                rb = rpool.tile([DH, NT], F32, tag="rb")
                nc.sync.dma_start(out=rb[:], in_=rdB[:].to_broadcast((DH, NT)))
                rbs.append(rb)
            for t in range(2):
                nc.vector.tensor_tensor(
                    cT[cx_hc][cx_hp : cx_hp + DH, t * NT : (t + 1) * NT],
                    cus[t][0:DH, :],
                    rbs[t][:],
                    AL.mult,
                )
            return eT

        def fpre(m, aps):
            """pre-accumulate F-proj chunk m over kc 0..6 (cT[7] pending)."""
            for t in range(2):
                for kc in range(KC - 1):
                    nc.tensor.matmul(
                        aps[t],
                        cT[kc][:, m * P : (m + 1) * P],
                        wf_t[kc][:, t * NT : (t + 1) * NT],
                        start=(kc == 0),
                        stop=False,
                    )
            return aps

        # ---- software-pipelined emission: per window, the next chunk's qk
        # runs as a contiguous PE-bound block (letting ScalarE drain the exp
        # backlog), then scores interleave with the previous heads' ctx.
        emit_qk(0)
        eTs = {0: emit_scores(0), 1: emit_scores(1)}
        psF = {}
        for hc in range(KC):
            if hc + 1 < KC:
                emit_qk(hc + 1)
            h_new = 2 * hc + 2
            eTn = emit_ilv(h_new if h_new < H else None, 2 * hc, eTs.pop(2 * hc))
            if eTn is not None:
                eTs[h_new] = eTn
            if hc == KC - 1:
                # pre-accumulate F-proj m=0 over kc 0..6 while the last
                # head's ctx (which produces cT[7]) is still in flight, so
                # the F-phase doesn't stall on the final denominator chain.
                psF[0] = fpre(
                    0,
                    [mmps.tile([P, NT], F32, tag="mm", name=f"po0_{t}")[:] for t in range(2)],
                )
            h_new = 2 * hc + 3
            eTn = emit_ilv(h_new if h_new < H else None, 2 * hc + 1, eTs.pop(2 * hc + 1))
            if eTn is not None:
                eTs[h_new] = eTn
            if hc == KC - 1:
                # ctx(15)'s psum is evacuated ~0.7us after its matmuls; the
                # freed cpool tiles (and the long-idle spool tiles, sliced
                # into two 1-bank groups each) let m=1..3 pre-accumulate
                # during the final denominator-chain latency.
                psF[1] = fpre(
                    1,
                    [cpool.tile([P, NT], F32, tag="cx", name=f"po1_{t}")[:] for t in range(2)],
                )
                for m in (2, 3):
                    sp = spool.tile([P, L], F32, tag="sc", name=f"po{m}")
                    psF[m] = fpre(m, [sp[:, 0:NT], sp[:, NT : 2 * NT]])

        # ---- output projection + residual + layernorm, per l-chunk; bf16
        # outputs DMA out per half so the traffic overlaps the F-proj
        # stream.  ScalarE only does Sqrt here (exp stream just ended);
        # psum evacuation on DVE, squares on GPSIMD.
        for m in range(MT):
            xn_t = xpool.tile([P, D], BF16, tag="xn")
            nc.sync.dma_start(out=xn_t[:], in_=xn_e[m * P : (m + 1) * P, :])
            att = opool.tile([P, D], BF16, tag="att")
            for t in range(2):
                if m in psF:
                    ps = psF[m][t][:]
                    kcs = [KC - 1]
                else:
                    pool = mmps if t == 0 else cpool
                    tag = "mm" if t == 0 else "cx"
                    ps = pool.tile([P, NT], F32, tag=tag, name=f"po{m}_{t}")[:]
                    kcs = list(range(KC))
                for kc in kcs:
                    nc.tensor.matmul(
                        ps,
                        cT[kc][:, m * P : (m + 1) * P],
                        wf_t[kc][:, t * NT : (t + 1) * NT],
                        start=(kc == 0),
                        stop=False,
                    )
                nc.tensor.matmul(
                    ps,
                    ones_r[:],
                    bfr[:, t * NT : (t + 1) * NT],
                    start=False,
                    stop=True,
                )
                nc.scalar.activation(att[:, t * NT : (t + 1) * NT], ps, AF.Copy)
                nc.sync.dma_start(
                    out=out_e[1, m * P : (m + 1) * P, t * NT : (t + 1) * NT],
                    in_=att[:, t * NT : (t + 1) * NT],
                )
            # LN epilogue
            res = opool.tile([P, D], BF16, tag="res")
            ssum = stat.tile([P, 1], F32, tag="ss")
            nc.vector.scalar_tensor_tensor(
                res[:], att[:], 1.0, xn_t[:], AL.mult, AL.add, accum_out=ssum[:]
            )
            # sq's tensor output is discarded (only the accum row-sum is
            # used) — write it over xn_t, which the res-add just consumed;
            # by the F-phase the exp stream is over so ScalarE has headroom.
            sqs = stat.tile([P, 1], F32, tag="sqs")
            nc.scalar.activation(xn_t[:], res[:], AF.Square, accum_out=sqs[:])
            mu = stat.tile([P, 1], F32, tag="mu")
            nc.vector.tensor_scalar_mul(mu[:], ssum[:], 1.0 / D)
            msq = stat.tile([P, 1], F32, tag="msq")
            nc.vector.tensor_scalar(msq[:], mu[:], mu[:], None, AL.mult)
            var = stat.tile([P, 1], F32, tag="var")
            nc.vector.tensor_scalar(var[:], sqs[:], 1.0 / D, msq[:], AL.mult, AL.subtract)
            sd = stat.tile([P, 1], F32, tag="sd")
            nc.scalar.activation(sd[:], var[:], AF.Sqrt, bias=epsb[:])
            inv = stat.tile([P, 1], F32, tag="inv")
            nc.vector.reciprocal(inv[:], sd[:])
            if apply_affine:
                fin = opool.tile([P, D], F32, tag="fin")
                nc.vector.tensor_scalar(fin[:], res[:], mu[:], inv[:], AL.subtract, AL.mult)
                nc.vector.scalar_tensor_tensor(fin[:], fin[:], 1.0, gmb[:], AL.mult, AL.mult)
                finb = opool.tile([P, D], BF16, tag="finb")
                nc.vector.tensor_tensor(finb[:], fin[:], btb[:], AL.add)
            else:
                finb = opool.tile([P, D], BF16, tag="finb")
                nc.vector.tensor_scalar(finb[:], res[:], mu[:], inv[:], AL.subtract, AL.mult)
            nc.sync.dma_start(out=out_e[0, m * P : (m + 1) * P, :], in_=finb[:])

    _split_excess_waits(nc)
    return nc


def prepare_in_maps(inputs):
    x = np.asarray(inputs["x"], np.float32)
    xr = x.reshape(B, L, DIL, D).transpose(0, 2, 1, 3).reshape(NCORES, L, D)
    Wf = np.asarray(inputs["Wf"], np.float32)
    # softmax weights sum to 1, so ctx(v + bv) = ctx(v) + bv; fold bv
    # through the output projection into its bias.
    bf_eff = np.asarray(inputs["bv"], np.float32) @ Wf.T + np.asarray(
        inputs["bf"], np.float32
    )
    shared = {
        "wqT": (np.asarray(inputs["Wq"], np.float32).T * SCALE).astype(BF16_NP),
        "wkT": np.asarray(inputs["Wk"], np.float32).T.astype(BF16_NP),
        "wvT": np.asarray(inputs["Wv"], np.float32).T.astype(BF16_NP),
        "wfT": Wf.T.astype(BF16_NP),
        "bqc": np.ascontiguousarray(
            (np.asarray(inputs["bq"], np.float32) * SCALE).reshape(MT, P).T
        ),
        "bkc": np.ascontiguousarray(
            np.asarray(inputs["bk"], np.float32).reshape(MT, P).T
        ),
        "bfh": bf_eff.astype(BF16_NP),
        "gam": np.ascontiguousarray(inputs["gamma"], dtype=np.float32),
        "bet": np.ascontiguousarray(inputs["beta"], dtype=np.float32),
    }
    maps = []
    for c in range(NCORES):
        xs = np.ascontiguousarray(xr[c])
        m = dict(shared)
        m["xT"] = xs.T.astype(BF16_NP)
        m["xn"] = xs.astype(BF16_NP)
        maps.append(m)
    return maps


def gather_outputs(results):
    outs = np.stack(
        [np.asarray(results[c]["out"]).astype(np.float32) for c in range(NCORES)]
    )
    final = outs[:, 0].reshape(B, DIL, L, D).transpose(0, 2, 1, 3).reshape(B, S, D)
    atted = outs[:, 1].reshape(B, DIL, L, D).transpose(0, 2, 1, 3).reshape(B, S, D)
    return np.ascontiguousarray(final), np.ascontiguousarray(atted)


_GRAPHS = {}


def get_graph(apply_affine=False):
    if apply_affine not in _GRAPHS:
        _GRAPHS[apply_affine] = build_graph(apply_affine)
    return _GRAPHS[apply_affine]


def run(inputs, trace=False, **kw):
    # gamma/beta are fixed to ones/zeros by the reference's setup_inputs;
    # only emit the affine LN ops if they are actually non-identity.
    apply_affine = not (
        np.all(np.asarray(inputs["gamma"]) == 1.0)
        and np.all(np.asarray(inputs["beta"]) == 0.0)
    )
    nc = get_graph(apply_affine)
    maps = prepare_in_maps(inputs)
    res = run_bass_kernel_spmd(nc, maps, core_ids=list(range(NCORES)), trace=trace, **kw)
    return gather_outputs(res.results), res


def kernel(**inputs):
    (final, atted), _ = run(inputs, trace=False)
    return final, atted
